# revision 1
# baseline (speedup 1.0000x reference)
"""Trainium2 Bass kernel for DoubleHeadRNN (two independent GRUs over the same input).

Problem: x [64, 1024, 512]; two Keras-style GRUCells (reset_after=True) with
H=1024, T=1024 steps; returns (h_last_head0, h_last_head1).

Strategy (v2): one head per core (cores 0/1 produce the two heads; the SPMD
program is identical on all 8 cores). Per step the fused projection
g = [x_t; h] @ [W; U] runs as PE matmuls with h kept transposed (regenerated
each step by PE transposes). The candidate gate needs xh and hh separately
(h_cand = tanh(xh + r*hh)), so PSUM keeps [zneg | r | xh | hh] regions.
z columns are negated on host so one sigmoid yields zneg = 1-z directly:
    h_new = h + zneg * (cand - h)

Performance structure: the PE array is column-split into two concurrent
32-col-group tiles (tile_position=(0,0) and (0,64)); tile `t` computes a
*different* 256-wide slice of the H columns, so no partial-combination is
needed and every ACT/DVE gate op runs on all 128 partitions
(parts 0-63 = batch for tile0's slice, parts 64-127 = batch for tile1's).
H is processed in two halves per step (psum [128, 1024] = 2 banks, bufs=2
so halves and steps pipeline). All column/row permutations that this
storage order implies are folded into the host-side weight layout.

Storage order: H-natural index n = 512*h + 256*t + w lives at
h_cur[64*t + b, 256*h + w] (h = half, t = col-tile).
"""

import os
import numpy as np
from contextlib import ExitStack

B, T, D, H = 64, 1024, 512, 1024
KC = (D + H) // 128  # 12 K-chunks of the fused contraction
NCORES = 8

_cache = {}


def _build(n_steps, bf16=False):
    import concourse.bass as bass
    import concourse.tile as tile
    from concourse import bacc, mybir

    f32 = mybir.dt.float32
    r32 = mybir.dt.float32r
    # float32r: same 4-byte storage, PE streams 1 cycle/row vs fp32's 4.
    # All matmul-feeding tensors (xt, wu, hT) are declared float32r; the
    # hT copy from psum performs the required fp32r rounding.
    mdt = mybir.dt.bfloat16 if bf16 else r32
    AF = mybir.ActivationFunctionType

    def rc(ap):
        # PE runs fp32 matmuls at 4 cycles/row but float32r (same 4-byte
        # storage, reduced-precision multiply) at 1 cycle/row for N>=256.
        return ap.bitcast(r32)

    nc = bacc.Bacc(
        "TRN2", target_bir_lowering=False, debug=False, num_devices=NCORES
    )
    xt_d = nc.dram_tensor("xt", [n_steps * 128, 256], mdt, kind="ExternalInput").ap()
    wu_d = nc.dram_tensor("wu", [KC * 128, 3072], mdt, kind="ExternalInput").ap()
    id_d = nc.dram_tensor("ident", [128, 64], f32, kind="ExternalInput").ap()
    out_d = nc.dram_tensor("out", [64, 1024], f32, kind="ExternalOutput").ap()

    with tile.TileContext(nc) as tc, ExitStack() as ctx:
        const = ctx.enter_context(tc.tile_pool(name="const", bufs=1))
        state = ctx.enter_context(tc.tile_pool(name="state", bufs=1))
        xpool = ctx.enter_context(tc.tile_pool(name="xin", bufs=4))
        gates = ctx.enter_context(tc.tile_pool(name="gates", bufs=3))
        ppool = ctx.enter_context(tc.tile_pool(name="psum", bufs=2, space="PSUM"))
        xpsum = ctx.enter_context(tc.tile_pool(name="psumX", bufs=1, space="PSUM"))
        tpool = ctx.enter_context(tc.tile_pool(name="psumT", bufs=1, space="PSUM"))

        # --- persistent SBUF ---
        wu_s = const.tile([128, KC * 3072], mdt, tag="wu")
        for c in range(KC):
            nc.sync.dma_start(
                wu_s[:, c * 3072 : (c + 1) * 3072],
                wu_d[c * 128 : (c + 1) * 128, :],
            )
        ident = const.tile([128, 64], f32, tag="ident")
        nc.sync.dma_start(ident[:], id_d[:])

        # h state, parity pairs ([128, 512] storage order, see module docstring)
        h_cur = [state.tile([64, 1024], f32, tag=f"hcur{p}", name=f"hcur{p}") for p in range(2)]
        hT = [state.tile([128, 512], mdt, tag=f"hT{p}", name=f"hT{p}") for p in range(2)]
        nc.vector.memset(h_cur[0][:], 0.0)
        nc.vector.memset(hT[0][:].bitcast(f32), 0.0)

        def step(iv, p):
            """One GRU step reading state parity p, writing parity 1-p."""
            h_in, hT_in = h_cur[p], hT[p]
            h_out, hT_out = h_cur[1 - p], hT[1 - p]

            xt_t = xpool.tile([128, 256], mdt, tag="xt")
            nc.sync.dma_start(xt_t[:], xt_d[bass.ds(iv * 128, 128), :])

            h_new = h_out

            for hf in range(2):  # halves of H
                # psum ps [64, 1536]: [zneg 512 | r 512 | hh 512]; xh separate
                ps = ppool.tile([64, 1536], f32, tag="ps")
                xh = xpsum.tile([64, 512], f32, tag="xh")
                for c in range(KC):
                    lhsT = (
                        xt_t[:, c * 64 : (c + 1) * 64]
                        if c < 4
                        else hT_in[:, (c - 4) * 64 : (c - 3) * 64]
                    )
                    wb = c * 3072 + hf * 512
                    nc.tensor.matmul(
                        ps[:, 0:512], lhsT, wu_s[:, wb : wb + 512],
                        start=(c == 0), stop=(c == KC - 1), skip_group_check=True,
                    )
                    nc.tensor.matmul(
                        ps[:, 512:1024], lhsT, wu_s[:, wb + 1024 : wb + 1536],
                        start=(c == 0), stop=(c == KC - 1), skip_group_check=True,
                    )
                    if c < 4:
                        nc.tensor.matmul(
                            xh[:, 0:512], lhsT,
                            wu_s[:, wb + 2048 : wb + 2560],
                            start=(c == 0), stop=(c == 3), skip_group_check=True,
                        )
                    else:
                        nc.tensor.matmul(
                            ps[:, 1024:1536], lhsT,
                            wu_s[:, wb + 2048 : wb + 2560],
                            start=(c == 4), stop=(c == KC - 1), skip_group_check=True,
                        )

                zr = gates.tile([64, 1024], f32, tag="zr")
                nc.scalar.activation(zr[:], ps[:, 0:1024], AF.Sigmoid)
                t1 = gates.tile([64, 512], f32, tag="t1")
                nc.vector.tensor_mul(t1[:], zr[:, 512:1024], ps[:, 1024:1536])
                t2 = gates.tile([64, 512], f32, tag="t2")
                nc.vector.tensor_add(t2[:], t1[:], xh[:])
                cand = gates.tile([64, 512], f32, tag="cand")
                nc.scalar.activation(cand[:], t2[:], AF.Tanh)
                hs = h_in[:, hf * 512 : (hf + 1) * 512]
                d = gates.tile([64, 512], f32, tag="d")
                nc.vector.tensor_sub(d[:], cand[:], hs)
                e = gates.tile([64, 512], f32, tag="e")
                nc.vector.tensor_mul(e[:], zr[:, 0:512], d[:])
                nc.vector.tensor_add(h_new[:, hf * 512 : (hf + 1) * 512], hs, e[:])

            # update state: transpose h_new (== h_out) -> hT_out
            pt = tpool.tile([128, 512], f32, tag="pt")
            for k in range(8):
                nc.tensor.transpose(
                    pt[:, k * 64 : (k + 1) * 64],
                    h_new[:, k * 128 : (k + 1) * 128],
                    ident[0:64, :],
                )
            # split copy: chunks 0-3 land early so next step's first h-MMs
            # need not wait for half1's transposes
            nc.vector.tensor_copy(hT_out[:, 0:256], pt[:, 0:256])
            nc.vector.tensor_copy(hT_out[:, 256:512], pt[:, 256:512])

        with tc.For_i(0, n_steps, 4, hint_engines=(mybir.EngineType.PE,), staggered_reset=True) as i:
            step(i, 0)
            step(i + 1, 1)
            step(i + 2, 0)
            step(i + 3, 1)

        nc.sync.dma_start(out_d[:], h_cur[0][:])

    nc.compile()
    return nc


def _col_perm():
    """Natural column order: [zneg 1024 | r 1024 | hc 1024]."""
    return np.arange(3 * H, dtype=np.int64)


def _row_perm_u():
    """Natural U-row order (h stored unpermuted)."""
    return np.arange(H, dtype=np.int64)


_CPERM = _col_perm()
_RPERM = _row_perm_u()


def _host_prep(x, W, U, bf16=False):
    """Build xt / wu host-side arrays for one head."""
    n_steps = x.shape[1]
    xt = (
        x.transpose(1, 2, 0)                      # [T, D, B]
        .reshape(n_steps, 4, 128, B)              # [T, c, p, b]
        .transpose(0, 2, 1, 3)                    # [T, p, c, b]
        .reshape(n_steps * 128, 256)
        .astype(np.float32)
    )
    Wp = np.asarray(W, np.float32)[:, _CPERM]
    Up = np.asarray(U, np.float32)[_RPERM][:, _CPERM]
    wu = np.concatenate([Wp, Up], axis=0).copy()  # [1536, 3072]
    # negate z columns
    wu[:, 0:H] *= -1.0
    if bf16:
        import ml_dtypes
        xt = xt.astype(ml_dtypes.bfloat16)
        wu = wu.astype(ml_dtypes.bfloat16)
    return np.ascontiguousarray(xt), np.ascontiguousarray(wu)


def _unpermute_h(res):
    """h is stored in natural order now."""
    return np.asarray(res, np.float32)


def _run_spmd(nc, in_maps, n_timed=0):
    """Execute on the 8 axon cores via PJRT shard_map; keeps the jitted
    callable + device inputs resident so timed runs measure execution."""
    import time
    import jax
    from jax.sharding import Mesh, PartitionSpec
    from jax.experimental.shard_map import shard_map
    from concourse import bass2jax, mybir

    bass2jax.install_neuronx_cc_hook()
    n_cores = len(in_maps)

    in_names, out_names, out_avals = [], [], []
    partition_name = nc.partition_id_tensor.name if nc.partition_id_tensor else None
    for alloc in nc.m.functions[0].allocations:
        if not isinstance(alloc, mybir.MemoryLocationSet):
            continue
        name = alloc.memorylocations[0].name
        if alloc.kind == "ExternalInput":
            if name != partition_name:
                in_names.append(name)
        elif alloc.kind == "ExternalOutput":
            shape = tuple(alloc.tensor_shape)
            dtype = mybir.dt.np(alloc.dtype)
            out_avals.append(jax.core.ShapedArray(shape, dtype))
            out_names.append(name)
    n_params = len(in_names)
    n_outs = len(out_names)
    all_in = in_names + out_names
    if partition_name is not None:
        all_in.append(partition_name)

    def _body(*args):
        operands = list(args)
        if partition_name is not None:
            operands.append(bass2jax.partition_id_tensor())
        outs = bass2jax._bass_exec_p.bind(
            *operands,
            out_avals=tuple(out_avals),
            in_names=tuple(all_in),
            out_names=tuple(out_names),
            lowering_input_output_aliases=(),
            sim_require_finite=True,
            sim_require_nnan=True,
            nc=nc,
        )
        return tuple(outs)

    devices = jax.devices()[:n_cores]
    mesh = Mesh(np.asarray(devices), ("core",))
    in_specs = (PartitionSpec("core"),) * (n_params + n_outs)
    out_specs = (PartitionSpec("core"),) * n_outs
    sharded = jax.jit(
        shard_map(_body, mesh=mesh, in_specs=in_specs, out_specs=out_specs,
                  check_rep=False),
        keep_unused=True,
    )
    sharding = jax.sharding.NamedSharding(mesh, PartitionSpec("core"))

    def _stage(per_core_arrays):
        shards = []
        for c, arr in enumerate(per_core_arrays):
            sh = jax.device_put(np.asarray(arr), devices[c])
            sh.block_until_ready()
            shards.append(sh)
        a0 = np.asarray(per_core_arrays[0])
        gshape = (n_cores * a0.shape[0], *a0.shape[1:])
        return jax.make_array_from_single_device_arrays(gshape, sharding, shards)

    dev_in = [_stage([in_maps[c][nm] for c in range(n_cores)]) for nm in in_names]
    dev_zero = [
        _stage([np.zeros(av.shape, av.dtype) for _ in range(n_cores)])
        for av in out_avals
    ]
    for a in dev_in + dev_zero:
        a.block_until_ready()

    out_arrs = sharded(*dev_in, *dev_zero)
    jax.block_until_ready(out_arrs)

    best = None
    for _ in range(n_timed):
        t0 = time.perf_counter_ns()
        out_arrs = sharded(*dev_in, *dev_zero)
        jax.block_until_ready(out_arrs)
        dt = time.perf_counter_ns() - t0
        best = dt if best is None else min(best, dt)

    results = [
        {
            nm: np.asarray(out_arrs[i]).reshape(n_cores, *out_avals[i].shape)[c]
            for i, nm in enumerate(out_names)
        }
        for c in range(n_cores)
    ]
    return results, best


def _make_ident():
    id2 = np.zeros((128, 64), np.float32)
    for p in range(128):
        id2[p, p % 64] = 1.0
    return id2


def kernel(x, W0, U0, bi0, br0, W1, U1, bi1, br1):
    x = np.asarray(x, dtype=np.float32)
    assert all(
        not np.any(np.asarray(b)) for b in (bi0, br0, bi1, br1)
    ), "nonzero biases not supported by this kernel build"

    bf16 = bool(int(os.environ.get("GRU_BF16", "0")))
    n_steps = x.shape[1]
    key = (n_steps, bf16)
    if key not in _cache:
        _cache[key] = _build(n_steps, bf16=bf16)
    nc = _cache[key]

    xt, wu0 = _host_prep(x, np.asarray(W0), np.asarray(U0), bf16=bf16)
    _, wu1 = _host_prep(x[:, :1], np.asarray(W1), np.asarray(U1), bf16=bf16)
    ident = _make_ident()

    maps = []
    for core in range(NCORES):
        wu = wu0 if core % 2 == 0 else wu1
        maps.append({"xt": xt, "wu": wu, "ident": ident})

    n_timed = int(os.environ.get("GRU_TIMED_RUNS", "0"))
    results, best_ns = _run_spmd(nc, maps, n_timed=n_timed)
    kernel.last_exec_ns = best_ns
    out0 = _unpermute_h(results[0]["out"])
    out1 = _unpermute_h(results[1]["out"])
    return out0, out1


kernel.last_exec_ns = None



# revision 6
# speedup vs baseline: 1.2063x; 1.2063x over previous
"""Trainium2 Bass kernel for DoubleHeadRNN (two independent GRUs over the same input).

Problem: x [64, 1024, 512]; two Keras-style GRUCells (reset_after=True) with
H=1024, T=1024 steps; returns (h_last_head0, h_last_head1).

Strategy (v4): one head per core (cores 0/1 produce the two heads; the SPMD
program is identical on all 8 cores). Per step the fused projection
g = [x_t; h] @ [W; U] runs as PE matmuls with the weights as the moving
operand (N=512 streams, 12 K-chunks of the 1536-contraction) and h kept
transposed (regenerated each step by PE transposes). H is processed in two
halves per step (psum [64, 1536] = [zneg | r | hh] per half, bufs=2 so halves
and steps pipeline; xh separate). z columns are negated on host so one
sigmoid yields zneg = 1-z directly:  h_new = h + zneg * (cand - h)

v4 scheduling fixes vs the 21.2ms baseline:
  - The 8 transposes for step i's input state are emitted AFTER step i's
    x-only matmuls (not at the tail of step i-1), so the previous step's
    serial ACT/DVE chain overlaps ~3.4us of h-independent PE work instead
    of stalling the PE queue in front of the transposes.
  - Per half, h-matmul blocks run r first, then hh, then zneg, and the
    sigmoid is split (r before z) so the t1/t2/tanh chain starts as early
    as possible.
  - The xh (candidate x-part) matmul group is emitted after the h-blocks;
    it is single-buffered and this placement avoids stalling on the
    previous step's reader.
  - pt -> hT copies run on ScalarE (ACT), freeing DVE for the gate chain;
    hT is a single buffer (safe: its readers always precede the next
    writer in PE queue order).
"""

import os
import numpy as np
from contextlib import ExitStack

B, T, D, H = 64, 1024, 512, 1024
KC = (D + H) // 128  # 12 K-chunks of the fused contraction
NCORES = 8

_cache = {}


def _build(n_steps):
    import concourse.bass as bass
    import concourse.tile as tile
    from concourse import bacc, mybir

    f32 = mybir.dt.float32
    r32 = mybir.dt.float32r
    # float32r: same 4-byte storage, PE streams 1 cycle/row vs fp32's 4.
    mdt = r32
    AF = mybir.ActivationFunctionType

    nc = bacc.Bacc(
        "TRN2", target_bir_lowering=False, debug=False, num_devices=NCORES
    )
    xt_d = nc.dram_tensor("xt", [n_steps * 128, 256], mdt, kind="ExternalInput").ap()
    wu_d = nc.dram_tensor("wu", [KC * 128, 3072], mdt, kind="ExternalInput").ap()
    id_d = nc.dram_tensor("ident", [128, 64], f32, kind="ExternalInput").ap()
    out_d = nc.dram_tensor("out", [64, 1024], f32, kind="ExternalOutput").ap()

    with tile.TileContext(nc) as tc, ExitStack() as ctx:
        const = ctx.enter_context(tc.tile_pool(name="const", bufs=1))
        state = ctx.enter_context(tc.tile_pool(name="state", bufs=1))
        xpool = ctx.enter_context(tc.tile_pool(name="xin", bufs=4))
        gates = ctx.enter_context(tc.tile_pool(name="gates", bufs=3))
        ppool = ctx.enter_context(tc.tile_pool(name="psum", bufs=2, space="PSUM"))
        xpsum = ctx.enter_context(tc.tile_pool(name="psumX", bufs=1, space="PSUM"))
        tpool = ctx.enter_context(tc.tile_pool(name="psumT", bufs=1, space="PSUM"))

        # --- persistent SBUF ---
        wu_s = const.tile([128, KC * 3072], mdt, tag="wu")
        for c in range(KC):
            nc.sync.dma_start(
                wu_s[:, c * 3072 : (c + 1) * 3072],
                wu_d[c * 128 : (c + 1) * 128, :],
            )
        ident = const.tile([128, 64], f32, tag="ident")
        nc.sync.dma_start(ident[:], id_d[:])

        h_cur = [
            state.tile([64, 1024], f32, tag=f"hcur{p}", name=f"hcur{p}")
            for p in range(2)
        ]
        hT_s = state.tile([128, 512], mdt, tag="hT", name="hT")
        nc.vector.memset(h_cur[0][:], 0.0)

        def WU(c, b, hf):
            # weight columns for K-chunk c, gate block b (0=zneg,1=r,2=hc),
            # H-half hf. Host layout: natural [zneg 1024 | r 1024 | hc 1024].
            o = c * 3072 + b * 1024 + hf * 512
            return wu_s[:, o : o + 512]

        def step(iv, p):
            """One GRU step reading state parity p, writing parity 1-p."""
            h_in, h_out = h_cur[p], h_cur[1 - p]

            xt_t = xpool.tile([128, 256], mdt, tag="xt")
            nc.sync.dma_start(xt_t[:], xt_d[bass.ds(iv * 128, 128), :])

            ps = [
                ppool.tile([64, 1536], f32, tag="ps", name=f"ps{hf}")
                for hf in range(2)
            ]

            def xl(c):
                return xt_t[:, 64 * c : 64 * c + 64]

            def hl(m):
                return hT_s[:, 64 * m : 64 * m + 64]

            # --- Phase A: x-only contributions to r and zneg, both halves.
            for hf in range(2):
                for b in (1, 0):
                    for c in range(4):
                        nc.tensor.matmul(
                            ps[hf][:, b * 512 : b * 512 + 512],
                            xl(c),
                            WU(c, b, hf),
                            start=(c == 0),
                            stop=False,
                            skip_group_check=True,
                        )

            # --- Regenerate hT from h_in (written by the previous step's
            # tail). Emitted after Phase A so that chain overlaps PE work.
            pt = tpool.tile([128, 512], f32, tag="pt")
            for k in range(8):
                nc.tensor.transpose(
                    pt[:, k * 64 : (k + 1) * 64],
                    h_in[:, k * 128 : (k + 1) * 128],
                    ident[0:64, :],
                )
            nc.scalar.copy(hT_s[:, 0:256], pt[:, 0:256])
            nc.scalar.copy(hT_s[:, 256:512], pt[:, 256:512])

            # --- per half: h-matmuls (r, hh, zneg), xh, then gate math
            for hf in range(2):
                for b, grp_start in ((1, False), (2, True), (0, False)):
                    for m in range(8):
                        nc.tensor.matmul(
                            ps[hf][:, b * 512 : b * 512 + 512],
                            hl(m),
                            WU(4 + m, b, hf),
                            start=(grp_start and m == 0),
                            stop=(m == 7),
                            skip_group_check=True,
                        )
                xh = xpsum.tile([64, 512], f32, tag="xh")
                for c in range(4):
                    nc.tensor.matmul(
                        xh[:],
                        xl(c),
                        WU(c, 2, hf),
                        start=(c == 0),
                        stop=(c == 3),
                        skip_group_check=True,
                    )

                hs = h_in[:, hf * 512 : (hf + 1) * 512]
                rr = gates.tile([64, 512], f32, tag="rr")
                nc.scalar.activation(rr[:], ps[hf][:, 512:1024], AF.Sigmoid)
                t1 = gates.tile([64, 512], f32, tag="t1")
                nc.vector.tensor_mul(t1[:], rr[:], ps[hf][:, 1024:1536])
                t2 = gates.tile([64, 512], f32, tag="t2")
                nc.vector.tensor_add(t2[:], t1[:], xh[:])
                cand = gates.tile([64, 512], f32, tag="cand")
                nc.scalar.activation(cand[:], t2[:], AF.Tanh)
                zz = gates.tile([64, 512], f32, tag="zz")
                nc.scalar.activation(zz[:], ps[hf][:, 0:512], AF.Sigmoid)
                d = gates.tile([64, 512], f32, tag="d")
                nc.vector.tensor_sub(d[:], cand[:], hs)
                e = gates.tile([64, 512], f32, tag="e")
                nc.vector.tensor_mul(e[:], zz[:], d[:])
                nc.vector.tensor_add(
                    h_out[:, hf * 512 : (hf + 1) * 512], hs, e[:]
                )

        with tc.For_i(
            0, n_steps, 4, hint_engines=(mybir.EngineType.PE,), staggered_reset=True
        ) as i:
            step(i, 0)
            step(i + 1, 1)
            step(i + 2, 0)
            step(i + 3, 1)

        nc.sync.dma_start(out_d[:], h_cur[0][:])

    nc.compile()
    return nc


def _host_prep(x, W, U):
    """Build xt / wu host-side arrays for one head."""
    n_steps = x.shape[1]
    xt = (
        x.transpose(1, 2, 0)                      # [T, D, B]
        .reshape(n_steps, 4, 128, B)              # [T, c, p, b]
        .transpose(0, 2, 1, 3)                    # [T, p, c, b]
        .reshape(n_steps * 128, 256)
        .astype(np.float32)
    )
    Wp = np.asarray(W, np.float32)
    Up = np.asarray(U, np.float32)
    wu = np.concatenate([Wp, Up], axis=0).copy()  # [1536, 3072]
    # negate z columns so sigmoid yields 1-z directly
    wu[:, 0:H] *= -1.0
    return np.ascontiguousarray(xt), np.ascontiguousarray(wu)


def _unpermute_h(res):
    return np.asarray(res, np.float32)


def _run_spmd(nc, in_maps, n_timed=0):
    """Execute on the 8 axon cores via PJRT shard_map; keeps the jitted
    callable + device inputs resident so timed runs measure execution."""
    import time
    import jax
    from jax.sharding import Mesh, PartitionSpec
    from jax.experimental.shard_map import shard_map
    from concourse import bass2jax, mybir

    bass2jax.install_neuronx_cc_hook()
    n_cores = len(in_maps)

    in_names, out_names, out_avals = [], [], []
    partition_name = nc.partition_id_tensor.name if nc.partition_id_tensor else None
    for alloc in nc.m.functions[0].allocations:
        if not isinstance(alloc, mybir.MemoryLocationSet):
            continue
        name = alloc.memorylocations[0].name
        if alloc.kind == "ExternalInput":
            if name != partition_name:
                in_names.append(name)
        elif alloc.kind == "ExternalOutput":
            shape = tuple(alloc.tensor_shape)
            dtype = mybir.dt.np(alloc.dtype)
            out_avals.append(jax.core.ShapedArray(shape, dtype))
            out_names.append(name)
    n_params = len(in_names)
    n_outs = len(out_names)
    all_in = in_names + out_names
    if partition_name is not None:
        all_in.append(partition_name)

    def _body(*args):
        operands = list(args)
        if partition_name is not None:
            operands.append(bass2jax.partition_id_tensor())
        outs = bass2jax._bass_exec_p.bind(
            *operands,
            out_avals=tuple(out_avals),
            in_names=tuple(all_in),
            out_names=tuple(out_names),
            lowering_input_output_aliases=(),
            sim_require_finite=True,
            sim_require_nnan=True,
            nc=nc,
        )
        return tuple(outs)

    devices = jax.devices()[:n_cores]
    mesh = Mesh(np.asarray(devices), ("core",))
    in_specs = (PartitionSpec("core"),) * (n_params + n_outs)
    out_specs = (PartitionSpec("core"),) * n_outs
    sharded = jax.jit(
        shard_map(_body, mesh=mesh, in_specs=in_specs, out_specs=out_specs,
                  check_rep=False),
        keep_unused=True,
    )
    sharding = jax.sharding.NamedSharding(mesh, PartitionSpec("core"))

    def _stage(per_core_arrays):
        shards = []
        for c, arr in enumerate(per_core_arrays):
            sh = jax.device_put(np.asarray(arr), devices[c])
            sh.block_until_ready()
            shards.append(sh)
        a0 = np.asarray(per_core_arrays[0])
        gshape = (n_cores * a0.shape[0], *a0.shape[1:])
        return jax.make_array_from_single_device_arrays(gshape, sharding, shards)

    dev_in = [_stage([in_maps[c][nm] for c in range(n_cores)]) for nm in in_names]
    dev_zero = [
        _stage([np.zeros(av.shape, av.dtype) for _ in range(n_cores)])
        for av in out_avals
    ]
    for a in dev_in + dev_zero:
        a.block_until_ready()

    out_arrs = sharded(*dev_in, *dev_zero)
    jax.block_until_ready(out_arrs)

    best = None
    for _ in range(n_timed):
        t0 = time.perf_counter_ns()
        out_arrs = sharded(*dev_in, *dev_zero)
        jax.block_until_ready(out_arrs)
        dt = time.perf_counter_ns() - t0
        best = dt if best is None else min(best, dt)

    results = [
        {
            nm: np.asarray(out_arrs[i]).reshape(n_cores, *out_avals[i].shape)[c]
            for i, nm in enumerate(out_names)
        }
        for c in range(n_cores)
    ]
    return results, best


def _make_ident():
    id2 = np.zeros((128, 64), np.float32)
    for p in range(128):
        id2[p, p % 64] = 1.0
    return id2


def kernel(x, W0, U0, bi0, br0, W1, U1, bi1, br1):
    x = np.asarray(x, dtype=np.float32)
    assert all(
        not np.any(np.asarray(b)) for b in (bi0, br0, bi1, br1)
    ), "nonzero biases not supported by this kernel build"

    n_steps = x.shape[1]
    key = n_steps
    if key not in _cache:
        _cache[key] = _build(n_steps)
    nc = _cache[key]

    xt, wu0 = _host_prep(x, np.asarray(W0), np.asarray(U0))
    _, wu1 = _host_prep(x[:, :1], np.asarray(W1), np.asarray(U1))
    ident = _make_ident()

    maps = []
    for core in range(NCORES):
        wu = wu0 if core % 2 == 0 else wu1
        maps.append({"xt": xt, "wu": wu, "ident": ident})

    n_timed = int(os.environ.get("GRU_TIMED_RUNS", "0"))
    results, best_ns = _run_spmd(nc, maps, n_timed=n_timed)
    kernel.last_exec_ns = best_ns
    out0 = _unpermute_h(results[0]["out"])
    out1 = _unpermute_h(results[1]["out"])
    return out0, out1


kernel.last_exec_ns = None


# revision 17
# speedup vs baseline: 2.2782x; 1.8886x over previous
"""Trainium2 Bass kernel for DoubleHeadRNN (two independent GRUs over the same input).

Problem: x [64, 1024, 512]; two Keras-style GRUCells (reset_after=True) with
H=1024, T=1024 steps; returns (h_last_head0, h_last_head1).

Strategy (v5): one head per core (cores 0/1 produce the two heads; the SPMD
program is identical on all 8 cores). Per step the fused projection
g = [x_t; h] @ [W; U] runs as PE matmuls with the weights as the moving
operand (N=512 streams) and h kept transposed (regenerated each step by PE
transposes). H is processed in two halves (psum P [128, 1536] = [zneg | r |
xh] per half, bufs=2; hh separate). z columns are negated on host so one
sigmoid yields zneg = 1-z:  h_new = h + zneg * (cand - h)

v5 核心 trick — STEP-PAIRED x-projections: the x-part of two consecutive
steps shares ONE weight stream. The stationary is [x_t chunk | x_{t+1} chunk]
(M=128), so each wu x-column is streamed once per PAIR instead of once per
step (-2.56us/step of PE streaming). Outputs land on psum partitions 0-63
(step t) and 64-127 (step t+1) of the same banks:
  - step t's recurrent matmuls accumulate into partitions 0-63 with plain
    M=64 stationaries [hT chunk | zeros-implied? no: [chunk|Z] M=128];
  - step t+1's use zero-padded stationaries [Z | hT chunk] (M=128), adding
    zero to the finished step-t values (harmless) and the real h-part to
    partitions 64-127. No PE column tiling needed (walrus rejects it).
  - step t+1's gate ops read psum at base partition 64 directly: ScalarE
    activations and DVE ops with one shifted-PSUM operand are legal and
    were verified on hardware (cross-partition reads land on lanes 0-63).
Two hT tables exist: hTe blocks [chunk | Z], hTo blocks [Z | chunk]; each
step's transposes refresh only the table that step's matmuls read.

Scheduling (from the v4 trace work): transposes for step s are emitted after
the pair's x-matmuls so the previous step's serial ACT/DVE chain overlaps PE
work; per half the h-blocks run r first, then hh, then zneg; sigmoid is split
(r before z); pt->hT copies run on ScalarE; loop unrolled 8 steps to amortize
the per-iteration loop barrier (~3.9us).
"""

import os
import numpy as np
from contextlib import ExitStack

B, T, D, H = 64, 1024, 512, 1024
KC = (D + H) // 128  # 12 K-chunks of the fused contraction
NCORES = 8

_cache = {}


def _build(n_steps):
    import concourse.bass as bass
    import concourse.tile as tile
    from concourse import bacc, mybir

    f32 = mybir.dt.float32
    r32 = mybir.dt.float32r
    mdt = r32
    AF = mybir.ActivationFunctionType

    nc = bacc.Bacc(
        "TRN2", target_bir_lowering=False, debug=False, num_devices=NCORES
    )
    # pair-interleaved x: row j*128+p, col 128c+64s+b = x[b, 2j+s, 128c+p]
    xt_d = nc.dram_tensor("xt", [(n_steps // 2) * 128, 512], mdt, kind="ExternalInput").ap()
    wu_d = nc.dram_tensor("wu", [KC * 128, 3072], mdt, kind="ExternalInput").ap()
    id_d = nc.dram_tensor("ident", [128, 64], f32, kind="ExternalInput").ap()
    out_d = nc.dram_tensor("out", [64, 1024], f32, kind="ExternalOutput").ap()

    with tile.TileContext(nc) as tc, ExitStack() as ctx:
        const = ctx.enter_context(tc.tile_pool(name="const", bufs=1))
        state = ctx.enter_context(tc.tile_pool(name="state", bufs=1))
        xpool = ctx.enter_context(tc.tile_pool(name="xin", bufs=4))
        gates = ctx.enter_context(tc.tile_pool(name="gates", bufs=2))
        ppool = ctx.enter_context(tc.tile_pool(name="psum", bufs=2, space="PSUM"))
        xpsum = ctx.enter_context(tc.tile_pool(name="psumX", bufs=1, space="PSUM"))
        tpool = ctx.enter_context(tc.tile_pool(name="psumT", bufs=1, space="PSUM"))

        # --- persistent SBUF ---
        wu_s = const.tile([128, KC * 3072], mdt, tag="wu")
        for c in range(KC):
            nc.sync.dma_start(
                wu_s[:, c * 3072 : (c + 1) * 3072],
                wu_d[c * 128 : (c + 1) * 128, :],
            )
        ident = const.tile([128, 64], f32, tag="ident")
        nc.sync.dma_start(ident[:], id_d[:])

        h_cur = [
            state.tile([64, 1024], f32, tag=f"hcur{p}", name=f"hcur{p}")
            for p in range(2)
        ]
        # padded stationary tables: block m of hTe = [hT chunk m | zeros],
        # of hTo = [zeros | hT chunk m]
        hTe_lo = state.tile([128, 4, 128], mdt, tag="hTe_lo", name="hTe_lo")
        hTe_hi = state.tile([128, 4, 128], mdt, tag="hTe_hi", name="hTe_hi")
        hTo_lo = state.tile([128, 4, 128], mdt, tag="hTo_lo", name="hTo_lo")
        hTo_hi = state.tile([128, 4, 128], mdt, tag="hTo_hi", name="hTo_hi")
        hTe = (hTe_lo, hTe_hi)
        hTo = (hTo_lo, hTo_hi)
        nc.vector.memset(h_cur[0][:], 0.0)
        for tt in (*hTe, *hTo):
            nc.vector.memset(tt[:].bitcast(f32), 0.0)

        def WU(c, b, hf):
            # gate block b (0=zneg,1=r,2=hc), H-half hf; host column layout
            # is natural [zneg 1024 | r 1024 | hc 1024] with zneg negated.
            o = c * 3072 + b * 1024 + hf * 512
            return wu_s[:, o : o + 512]

        def pair(iv):
            """Steps t=iv (parity 0) and t+1 (parity 1)."""
            xt2 = xpool.tile([128, 512], mdt, tag="xt")
            nc.sync.dma_start(xt2[:], xt_d[bass.ds(iv * 64, 128), :])

            P = [
                ppool.tile([128, 1024], f32, tag="ps", name=f"ps{hf}")
                for hf in range(2)
            ]
            PX = [
                ppool.tile([128, 512], f32, tag="psxh", name=f"psxh{hf}")
                for hf in range(2)
            ]

            # --- pair x-matmuls: stationary [x_t | x_t+1] chunk (M=128)
            def xpair(hf):
                for b in (1, 0, 2):  # r, zneg, then xh (xh group completes)
                    dst = PX[hf][:, :] if b == 2 else P[hf][:, b * 512 : b * 512 + 512]
                    for c in range(4):
                        nc.tensor.matmul(
                            dst,
                            xt2[:, 128 * c : 128 * c + 128],
                            WU(c, b, hf),
                            start=(c == 0),
                            stop=(b == 2 and c == 3),
                            skip_group_check=True,
                        )

            xpair(0)

            def hmm(hT, b, hf, ms, s):
                col0 = 512 if b == 1 else 0
                for m in ms:
                    nc.tensor.matmul(
                        P[hf][:, col0 : col0 + 512],
                        hT[m // 4][:, m % 4, :],
                        WU(4 + m, b, hf),
                        start=False,
                        stop=(s == 1 and m == 7),
                        skip_group_check=True,
                    )

            def hhmm(hT, hf, coff, dst, ms=range(8), cont=False):
                for m in ms:
                    nc.tensor.matmul(
                        dst,
                        hT[m // 4][:, m % 4, coff : coff + 64],
                        WU(4 + m, 2, hf),
                        start=(m == 0 and not cont),
                        stop=(m == 7),
                        skip_group_check=True,
                    )

            def gate_math(s, hf, h_in, h_out, hp):
                base = 64 * s
                hs = h_in[:, hf * 512 : (hf + 1) * 512]
                rr = gates.tile([64, 512], f32, tag="rr")
                nc.scalar.activation(
                    rr[:], P[hf][base : base + 64, 512:1024], AF.Sigmoid
                )
                t1 = gates.tile([64, 512], f32, tag="t1")
                nc.vector.tensor_mul(t1[:], rr[:], hp)
                t2 = gates.tile([64, 512], f32, tag="t2")
                nc.vector.tensor_add(
                    t2[:], t1[:], PX[hf][base : base + 64, :]
                )
                cand = gates.tile([64, 512], f32, tag="cand")
                nc.scalar.activation(cand[:], t2[:], AF.Tanh)
                zz = gates.tile([64, 512], f32, tag="zz")
                nc.scalar.activation(
                    zz[:], P[hf][base : base + 64, 0:512], AF.Sigmoid
                )
                d = gates.tile([64, 512], f32, tag="d")
                nc.vector.tensor_sub(d[:], cand[:], hs)
                e = gates.tile([64, 512], f32, tag="e")
                nc.vector.tensor_mul(e[:], zz[:], d[:])
                nc.vector.tensor_add(
                    h_out[:, hf * 512 : (hf + 1) * 512], hs, e[:]
                )

            # ---- step t (parity 0): reads partitions 0-63 of P
            h_in, h_out = h_cur[0], h_cur[1]
            pt = tpool.tile([128, 8, 64], f32, tag="pt")
            for k in range(8):
                nc.tensor.transpose(
                    pt[:, k, :], h_in[:, k * 128 : (k + 1) * 128], ident[0:64, :]
                )
            nc.scalar.copy(hTe_lo[:, :, 0:64], pt[:, 0:4, :])
            nc.scalar.copy(hTe_hi[:, :, 0:64], pt[:, 4:8, :])
            xpair(1)  # fills the PE while the hTe copies land

            hmm(hTe, 1, 0, range(8), 0)               # r(h0)
            hp0 = xpsum.tile([64, 512], f32, tag="hp", name="hp0a")
            hhmm(hTe, 0, 0, hp0[:])                   # hh(h0)
            hmm(hTe, 0, 0, range(8), 0)               # zneg(h0)
            gate_math(0, 0, h_in, h_out, hp0[:])
            hmm(hTe, 1, 1, range(8), 0)               # r(h1)
            hp0b = xpsum.tile([64, 512], f32, tag="hp", name="hp0b")
            hhmm(hTe, 1, 0, hp0b[:])                  # hh(h1)
            # odd step's first transpose block: needs only h_out[:, 0:512]
            # (written by gate_math(0,0)), ready by now; zneg(h1) fills the
            # PE while the hTo copy lands.
            pt1 = tpool.tile([128, 8, 64], f32, tag="pt")
            for k in range(4):
                nc.tensor.transpose(
                    pt1[:, k, :], h_out[:, k * 128 : (k + 1) * 128], ident[0:64, :]
                )
            nc.vector.tensor_copy(hTo_lo[:, :, 64:128], pt1[:, 0:4, :])
            hmm(hTe, 0, 1, range(8), 0)               # zneg(h1)
            gate_math(0, 1, h_in, h_out, hp0b[:])

            # ---- step t+1 (parity 1): reads partitions 64-127 of P.
            h_in, h_out = h_cur[1], h_cur[0]
            for hf in range(2):
                hmm(hTo, 1, hf, range(4), 1)          # r m0-3
                hmm(hTo, 0, hf, range(4), 1)          # zneg m0-3
            for k in range(4, 8):
                nc.tensor.transpose(
                    pt1[:, k, :], h_in[:, k * 128 : (k + 1) * 128], ident[0:64, :]
                )
            nc.vector.tensor_copy(hTo_hi[:, :, 64:128], pt1[:, 4:8, :])
            # hh(h0) m0-3 fills the PE while the copy lands
            hp1 = xpsum.tile([64, 512], f32, tag="hp", name="hp1a")
            hhmm(hTo, 0, 64, hp1[:], ms=range(4))
            hmm(hTo, 1, 0, range(4, 8), 1)            # r(h0) m4-7 -> rr chain
            hhmm(hTo, 0, 64, hp1[:], ms=range(4, 8), cont=True)
            hmm(hTo, 0, 0, range(4, 8), 1)            # zneg(h0) m4-7
            gate_math(1, 0, h_in, h_out, hp1[:])
            hmm(hTo, 1, 1, range(4, 8), 1)            # r(h1) m4-7
            hp2 = tpool.tile([128, 8, 64], f32, tag="pt", name="hp2a")
            hhmm(hTo, 1, 64, hp2[0:64, :, :])
            hmm(hTo, 0, 1, range(4, 8), 1)            # zneg(h1) m4-7
            gate_math(1, 1, h_in, h_out, hp2[0:64, :, :])

        unroll = 16 if n_steps % 16 == 0 else (8 if n_steps % 8 == 0 else 2)
        with tc.For_i(
            0, n_steps, unroll, hint_engines=(mybir.EngineType.PE,),
            staggered_reset=True,
        ) as i:
            for j in range(unroll // 2):
                pair(i + 2 * j)

        nc.sync.dma_start(out_d[:], h_cur[0][:])

    nc.compile()
    return nc


def _host_prep(x, W, U):
    """Build pair-interleaved xt / wu host-side arrays for one head."""
    n_steps = x.shape[1]
    xt = (
        x.transpose(1, 2, 0)                      # [T, D, B]
        .reshape(n_steps // 2, 2, 4, 128, B)      # [j, s, c, p, b]
        .transpose(0, 3, 2, 1, 4)                 # [j, p, c, s, b]
        .reshape((n_steps // 2) * 128, 512)
        .astype(np.float32)
    )
    Wp = np.asarray(W, np.float32)
    Up = np.asarray(U, np.float32)
    wu = np.concatenate([Wp, Up], axis=0).copy()  # [1536, 3072]
    # negate z columns so sigmoid yields 1-z directly
    wu[:, 0:H] *= -1.0
    return np.ascontiguousarray(xt), np.ascontiguousarray(wu)


def _unpermute_h(res):
    return np.asarray(res, np.float32)


def _run_spmd(nc, in_maps, n_timed=0):
    """Execute on the 8 axon cores via PJRT shard_map; keeps the jitted
    callable + device inputs resident so timed runs measure execution."""
    import time
    import jax
    from jax.sharding import Mesh, PartitionSpec
    from jax.experimental.shard_map import shard_map
    from concourse import bass2jax, mybir

    bass2jax.install_neuronx_cc_hook()
    n_cores = len(in_maps)

    in_names, out_names, out_avals = [], [], []
    partition_name = nc.partition_id_tensor.name if nc.partition_id_tensor else None
    for alloc in nc.m.functions[0].allocations:
        if not isinstance(alloc, mybir.MemoryLocationSet):
            continue
        name = alloc.memorylocations[0].name
        if alloc.kind == "ExternalInput":
            if name != partition_name:
                in_names.append(name)
        elif alloc.kind == "ExternalOutput":
            shape = tuple(alloc.tensor_shape)
            dtype = mybir.dt.np(alloc.dtype)
            out_avals.append(jax.core.ShapedArray(shape, dtype))
            out_names.append(name)
    n_params = len(in_names)
    n_outs = len(out_names)
    all_in = in_names + out_names
    if partition_name is not None:
        all_in.append(partition_name)

    def _body(*args):
        operands = list(args)
        if partition_name is not None:
            operands.append(bass2jax.partition_id_tensor())
        outs = bass2jax._bass_exec_p.bind(
            *operands,
            out_avals=tuple(out_avals),
            in_names=tuple(all_in),
            out_names=tuple(out_names),
            lowering_input_output_aliases=(),
            sim_require_finite=True,
            sim_require_nnan=True,
            nc=nc,
        )
        return tuple(outs)

    devices = jax.devices()[:n_cores]
    mesh = Mesh(np.asarray(devices), ("core",))
    in_specs = (PartitionSpec("core"),) * (n_params + n_outs)
    out_specs = (PartitionSpec("core"),) * n_outs
    sharded = jax.jit(
        shard_map(_body, mesh=mesh, in_specs=in_specs, out_specs=out_specs,
                  check_rep=False),
        keep_unused=True,
    )
    sharding = jax.sharding.NamedSharding(mesh, PartitionSpec("core"))

    def _stage(per_core_arrays):
        shards = []
        for c, arr in enumerate(per_core_arrays):
            sh = jax.device_put(np.asarray(arr), devices[c])
            sh.block_until_ready()
            shards.append(sh)
        a0 = np.asarray(per_core_arrays[0])
        gshape = (n_cores * a0.shape[0], *a0.shape[1:])
        return jax.make_array_from_single_device_arrays(gshape, sharding, shards)

    dev_in = [_stage([in_maps[c][nm] for c in range(n_cores)]) for nm in in_names]
    dev_zero = [
        _stage([np.zeros(av.shape, av.dtype) for _ in range(n_cores)])
        for av in out_avals
    ]
    for a in dev_in + dev_zero:
        a.block_until_ready()

    out_arrs = sharded(*dev_in, *dev_zero)
    jax.block_until_ready(out_arrs)

    best = None
    for _ in range(n_timed):
        t0 = time.perf_counter_ns()
        out_arrs = sharded(*dev_in, *dev_zero)
        jax.block_until_ready(out_arrs)
        dt = time.perf_counter_ns() - t0
        best = dt if best is None else min(best, dt)

    results = [
        {
            nm: np.asarray(out_arrs[i]).reshape(n_cores, *out_avals[i].shape)[c]
            for i, nm in enumerate(out_names)
        }
        for c in range(n_cores)
    ]
    return results, best


def _make_ident():
    id2 = np.zeros((128, 64), np.float32)
    for p in range(128):
        id2[p, p % 64] = 1.0
    return id2


def kernel(x, W0, U0, bi0, br0, W1, U1, bi1, br1):
    x = np.asarray(x, dtype=np.float32)
    assert all(
        not np.any(np.asarray(b)) for b in (bi0, br0, bi1, br1)
    ), "nonzero biases not supported by this kernel build"

    n_steps = x.shape[1]
    key = n_steps
    if key not in _cache:
        _cache[key] = _build(n_steps)
    nc = _cache[key]

    xt, wu0 = _host_prep(x, np.asarray(W0), np.asarray(U0))
    _, wu1 = _host_prep(x[:, :2], np.asarray(W1), np.asarray(U1))
    ident = _make_ident()

    maps = []
    for core in range(NCORES):
        wu = wu0 if core % 2 == 0 else wu1
        maps.append({"xt": xt, "wu": wu, "ident": ident})

    n_timed = int(os.environ.get("GRU_TIMED_RUNS", "0"))
    results, best_ns = _run_spmd(nc, maps, n_timed=n_timed)
    kernel.last_exec_ns = best_ns
    out0 = _unpermute_h(results[0]["out"])
    out1 = _unpermute_h(results[1]["out"])
    return out0, out1


kernel.last_exec_ns = None
